# revision 6
# baseline (speedup 1.0000x reference)
"""Trainium2 Bass kernel for nn_AttentionLayer (B=4, C=256, N=4096, CR=32).

Sharding: 8 cores = (batch b in 0..3) x (query-half ih in 0..1).
Each core receives the full x[b] (for keys/values) plus its query half,
computes out[b][:, ih*2048:(ih+1)*2048], host reassembles.

Per-core algorithm (all matmuls run in (128,128) PE tile mode):
  - stacked 1x1 conv  [Wk; Wq; bv@Wk] @ x -> g (keys), h (values),
    gbv row (query-bias correction, see below)
  - f = Wv @ xq + bv  (queries, own half only)
  - scores s^T[j,i] = sum_c g_aug[c,j] * f_aug[c,i] with augmented
    contraction row so s already contains the query-bias term:
       f_aug = [f; 1; 0...], g_aug = [g; gbv; 0...]  (K padded to 128)
    Key bias bk cancels in softmax (constant over j) and is dropped.
    Value bias bq is folded into the output conv bias on the host.
  - exp on ACT (no max subtraction needed: |s| <~ 40, fits fp32/bf16 range)
  - num/den via one accumulating matmul with H' = [h^T, 1, 0...] as lhsT
  - reciprocal(den) broadcast via PE ones-matmul, num * rden -> att
  - out = (gamma*Wo) @ att + (gamma*(Wo@bq + bo)) + xq  (bias via an
    exact-ones row in att and a bias row in the weight matrix)
"""

import numpy as np

B, C, N = 4, 256, 4096
CR = 32
NH = N // 2          # queries per core
G = 512              # i-group width
NCORES = 8

_CACHE = {}


def _dt():
    import concourse.mybir as mybir
    return mybir.dt


def build_program():
    """Build the (shared, SPMD) Bass program. Returns compiled nc."""
    import concourse.bacc as bacc
    import concourse.mybir as mybir
    from concourse.tile import TileContext

    dt = mybir.dt
    f32 = dt.float32
    bf16 = dt.bfloat16
    Exp = mybir.ActivationFunctionType.Exp
    add = mybir.AluOpType.add
    mult = mybir.AluOpType.mult

    nc = bacc.Bacc("TRN2", target_bir_lowering=False, debug=False,
                   num_devices=NCORES)

    # --- I/O ---
    xin = nc.dram_tensor("xin", [C, N], f32, kind="ExternalInput").ap()
    xqin = nc.dram_tensor("xqin", [C, NH], f32, kind="ExternalInput").ap()
    wght = nc.dram_tensor("wght", [128, 130], bf16, kind="ExternalInput").ap()
    wft = nc.dram_tensor("wft", [128, 256], bf16, kind="ExternalInput").ap()
    wot = nc.dram_tensor("wot", [128, 256], bf16, kind="ExternalInput").ap()
    bvv = nc.dram_tensor("bvv", [128, 1], f32, kind="ExternalInput").ap()
    e0 = nc.dram_tensor("e0", [128, 128], bf16, kind="ExternalInput").ap()
    idm = nc.dram_tensor("idm", [128, 128], bf16, kind="ExternalInput").ap()
    res = nc.dram_tensor("res", [C, NH], f32, kind="ExternalOutput").ap()

    NJT = N // 128            # 32 j-tiles
    NIG = NH // G             # 4 i-groups
    SUPERS = [4, 3, 4, 3, 4, 3, 4, 3, 4]   # j-tiles per super-tile (sum 32)
    assert sum(SUPERS) == NJT

    with TileContext(nc) as tc:
        with (
            tc.tile_pool(name="const", bufs=1) as cpool,
            tc.tile_pool(name="big", bufs=1) as bpool,
            tc.tile_pool(name="eb", bufs=2) as epool,
            tc.tile_pool(name="small", bufs=2) as spool,
            tc.tile_pool(name="resp", bufs=2) as rpool,
            tc.tile_pool(name="ps4", bufs=1, space="PSUM") as ps4,
            tc.tile_pool(name="ps3", bufs=1, space="PSUM") as ps3,
            tc.tile_pool(name="pso", bufs=1, space="PSUM") as pso,
        ):
            # --- load constants/weights ---
            wght_t = cpool.tile([128, 130], bf16)
            nc.sync.dma_start(wght_t[:], wght[:])
            wft_t = cpool.tile([128, 256], bf16)
            nc.sync.dma_start(wft_t[:], wft[:])
            wot_t = cpool.tile([128, 256], bf16)
            nc.sync.dma_start(wot_t[:], wot[:])
            bv_t = cpool.tile([128, 1], f32)
            nc.sync.dma_start(bv_t[:], bvv[:])
            e0_t = cpool.tile([128, 128], bf16)
            nc.sync.dma_start(e0_t[:], e0[:])
            idm_t = cpool.tile([128, 128], bf16)
            nc.sync.dma_start(idm_t[:], idm[:])

            # --- load x (full, 4 pieces) and xq (2 pieces) ---
            xsb = bpool.tile([128, 2 * N], f32)      # chunk-major fp32 x
            xr = xin.rearrange("(c p) (gp n) -> p gp c n", c=2, n=1024)
            xsb_v = xsb[:].rearrange("p (c gp n) -> p gp c n", c=2, n=1024)
            for gp in range(4):
                nc.sync.dma_start(xsb_v[:, gp], xr[:, gp])
            xqsb = bpool.tile([128, 2 * NH], f32)
            xqr = xqin.rearrange("(c p) (gp n) -> p gp c n", c=2, n=1024)
            xqsb_v = xqsb[:].rearrange("p (c gp n) -> p gp c n", c=2, n=1024)
            for gp in range(2):
                nc.sync.dma_start(xqsb_v[:, gp], xqr[:, gp])

            # --- bf16 casts ---
            xbf = bpool.tile([128, 2 * N], bf16)
            for gp in range(4):
                nc.vector.tensor_copy(
                    xbf[:].rearrange("p (c gp n) -> p gp c n", c=2,
                                     n=1024)[:, gp],
                    xsb_v[:, gp])
            xqbf = bpool.tile([128, 2 * NH], bf16)
            for gp in range(2):
                nc.vector.tensor_copy(
                    xqbf[:].rearrange("p (c gp n) -> p gp c n", c=2,
                                      n=1024)[:, gp],
                    xqsb_v[:, gp])

            # --- activation buffers ---
            f_aug = bpool.tile([128, NH], bf16)   # rows: f(32), ones(1), 0
            g_aug = bpool.tile([128, N], bf16)    # rows: g(32), gbv(1), 0
            h_sb = bpool.tile([128, N], bf16)     # rows: h(32)
            hpt = bpool.tile([128, N], bf16)      # [h^T;1;0..] per j-tile

            for lo in (32, 64, 96):
                nc.gpsimd.memset(f_aug[lo:lo + 32, :], 0.0)
                nc.gpsimd.memset(g_aug[lo:lo + 32, :], 0.0)
            nc.gpsimd.memset(f_aug[32:33, :], 1.0)
            nc.gpsimd.memset(hpt[:], 0.0)

            # --- stacked gh conv: [g; h; gbv] = W_gh @ x ---
            for grp in range(8):
                pool = ps4 if grp % 2 == 0 else ps3
                name = "s4" if grp % 2 == 0 else "s3"
                cps = pool.tile([128, G], f32, name=name)
                for c in range(2):
                    nc.tensor.matmul(
                        cps[0:65, :],
                        wght_t[:, c * 65:(c + 1) * 65],
                        xbf[:, c * N + grp * G: c * N + (grp + 1) * G],
                        start=(c == 0), stop=(c == 1))
                sl = slice(grp * G, (grp + 1) * G)
                nc.vector.tensor_copy(g_aug[0:32, sl], cps[0:32, :])
                nc.vector.tensor_copy(h_sb[0:32, sl], cps[32:64, :])
                nc.vector.tensor_copy(g_aug[32:33, sl], cps[64:65, :])

            # --- f conv (own query half): f = Wv @ xq + bv ---
            for fg in range(NIG):
                pool = ps4 if fg % 2 == 0 else ps3
                name = "s4" if fg % 2 == 0 else "s3"
                cps = pool.tile([128, G], f32, name=name)
                for c in range(2):
                    nc.tensor.matmul(
                        cps[:, :],
                        wft_t[:, c * 128:(c + 1) * 128],
                        xqbf[:, c * NH + fg * G: c * NH + (fg + 1) * G],
                        start=(c == 0), stop=(c == 1))
                nc.vector.tensor_scalar(
                    f_aug[0:32, fg * G:(fg + 1) * G], cps[0:32, :],
                    bv_t[0:32, 0:1], None, add)

            # --- transpose h into hpt ([h^T, ones, 0...] per j-tile) ---
            tps = pso.tile([128, 1024], bf16, name="o")
            for t in range(NJT):
                nc.tensor.transpose(
                    tps[:, t * 32:(t + 1) * 32],
                    h_sb[0:32, t * 128:(t + 1) * 128],
                    idm_t[0:32, 0:32])
            hpt_v = hpt[:].rearrange("p (t w) -> p t w", w=128)
            tps_v = tps[:].rearrange("p (t w) -> p t w", w=32)
            nc.vector.tensor_copy(hpt_v[:, :, 0:32], tps_v[:, :, :])
            nc.vector.memset(hpt_v[:, :, 32:33], 1.0)

            # --- main attention loop ---
            for g in range(NIG):
                po = pso.tile([128, G], f32, name="o")
                jt = 0
                for nt in SUPERS:
                    pool = ps4 if nt == 4 else ps3
                    name = "s4" if nt == 4 else "s3"
                    sps = pool.tile([128, nt * G], f32, name=name)
                    for t in range(nt):
                        nc.tensor.matmul(
                            sps[:, t * G:(t + 1) * G],
                            g_aug[:, (jt + t) * 128:(jt + t + 1) * 128],
                            f_aug[:, g * G:(g + 1) * G],
                            start=True, stop=True)
                    eb = epool.tile([128, 4 * G], bf16, name="eb")
                    nc.scalar.activation(eb[:, 0:nt * G], sps[:, 0:nt * G], Exp)
                    for t in range(nt):
                        nc.tensor.matmul(
                            po[:, :],
                            hpt[:, (jt + t) * 128:(jt + t + 1) * 128],
                            eb[:, t * G:(t + 1) * G],
                            start=(jt + t == 0), stop=(jt + t == NJT - 1))
                    jt += nt

                # reciprocal of den (row 32 of po), broadcast via PE
                rd = spool.tile([128, G], bf16, name="rd")
                nc.vector.memset(rd[:], 0.0)
                with nc.allow_low_precision(reason="bf16 softmax denom"):
                    nc.vector.reciprocal(rd[0:1, :], po[32:33, :])
                tail = ps3.tile([128, 1536], f32, name="s3")
                bc = tail[:, 0:G]
                pf = tail[:, G:G + 1024]
                nc.tensor.matmul(bc, e0_t[:, :], rd[:, :], start=True, stop=True)
                bcs = spool.tile([128, G], bf16, name="bcs")
                nc.vector.tensor_copy(bcs[:], bc)

                att = spool.tile([128, G], bf16, name="att")
                nc.vector.tensor_tensor(att[:], po[:, :], bcs[:], mult)
                nc.vector.memset(att[32:33, :], 1.0)

                # output conv (gamma*Wo, bias row) -> + xq residual
                for c in range(2):
                    nc.tensor.matmul(
                        pf[:, c * G:(c + 1) * G],
                        wot_t[:, c * 128:(c + 1) * 128],
                        att[:, :], start=True, stop=True)
                rt = rpool.tile([128, 1024], f32, name="rt")
                for c in range(2):
                    nc.vector.tensor_tensor(
                        rt[:, c * G:(c + 1) * G],
                        pf[:, c * G:(c + 1) * G],
                        xqsb[:, c * NH + g * G: c * NH + (g + 1) * G], add)
                out_v = res.rearrange("(c p) (gg n) -> p gg c n",
                                      c=2, n=G)[:, g]
                nc.sync.dma_start(
                    out_v, rt[:].rearrange("p (c n) -> p c n", c=2))

    nc.compile()
    return nc


def _host_prep(Wv, bv, Wk, bk, Wq, bq, Wo, bo, gamma):
    import ml_dtypes
    bf16 = ml_dtypes.bfloat16
    gam = float(np.asarray(gamma).reshape(-1)[0])

    # stacked gh conv weights: rows = [Wk(32); Wq(32); bv@Wk(1)]
    w_gh = np.concatenate([Wk, Wq, (bv @ Wk)[None, :]], axis=0)  # [65, 256]
    wght = np.zeros((128, 130), np.float32)
    for c in range(2):
        wght[:, c * 65:(c + 1) * 65] = w_gh.T[c * 128:(c + 1) * 128, :]

    wft = np.zeros((128, 256), np.float32)
    for c in range(2):
        wft[:, c * 128: c * 128 + 32] = Wv.T[c * 128:(c + 1) * 128, :]

    # output conv: lhsT rows k: k<32 -> gamma*Wo^T, k==33 -> bias row
    bof = gam * (Wo @ bq + bo)                                  # [256]
    wot = np.zeros((128, 256), np.float32)
    for c in range(2):
        wot[0:32, c * 128:(c + 1) * 128] = gam * Wo[c * 128:(c + 1) * 128, :].T
        wot[32, c * 128:(c + 1) * 128] = bof[c * 128:(c + 1) * 128]

    bvv = np.zeros((128, 1), np.float32)
    bvv[0:32, 0] = bv

    e0 = np.zeros((128, 128), np.float32)
    e0[0, :] = 1.0
    idm = np.eye(128, dtype=np.float32)

    return {
        "wght": wght.astype(bf16),
        "wft": wft.astype(bf16),
        "wot": wot.astype(bf16),
        "bvv": bvv,
        "e0": e0.astype(bf16),
        "idm": idm.astype(bf16),
    }


def kernel(**inputs):
    from concourse.bass_utils import run_bass_kernel_spmd

    x = np.asarray(inputs["x"], np.float32)
    consts = _host_prep(
        np.asarray(inputs["Wv"], np.float32),
        np.asarray(inputs["bv"], np.float32),
        np.asarray(inputs["Wk"], np.float32),
        np.asarray(inputs["bk"], np.float32),
        np.asarray(inputs["Wq"], np.float32),
        np.asarray(inputs["bq"], np.float32),
        np.asarray(inputs["Wo"], np.float32),
        np.asarray(inputs["bo"], np.float32),
        np.asarray(inputs["gamma"], np.float32),
    )

    if "nc" not in _CACHE:
        _CACHE["nc"] = build_program()
    nc = _CACHE["nc"]

    in_maps = []
    for core in range(NCORES):
        b, ih = core // 2, core % 2
        m = dict(consts)
        m["xin"] = np.ascontiguousarray(x[b])
        m["xqin"] = np.ascontiguousarray(x[b][:, ih * NH:(ih + 1) * NH])
        in_maps.append(m)

    r = run_bass_kernel_spmd(nc, in_maps, core_ids=list(range(NCORES)),
                             trace=False)
    out = np.empty((B, C, N), np.float32)
    for core in range(NCORES):
        b, ih = core // 2, core % 2
        out[b][:, ih * NH:(ih + 1) * NH] = r.results[core]["res"]
    return out


if __name__ == "__main__":
    nc = build_program()
    print("program built ok")


# revision 18
# speedup vs baseline: 1.4062x; 1.4062x over previous
"""Trainium2 Bass kernel for nn_AttentionLayer (B=4, C=256, N=4096, CR=32).

Sharding: 8 cores = (batch b in 0..3) x (query-half ih in 0..1).
Each core receives x[b] rotated so its own query half sits at columns
0..2047 (softmax is invariant to key order, so the rotation is exact);
it computes out[b][:, ih*2048:(ih+1)*2048] and the host reassembles.

Per-core algorithm (dtype float32r = TF32-class PE inputs, fp32 PSUM):
  - stacked 1x1 conv [Wk; bv@Wk; pad; Wq] @ x -> g (keys), gbv row
    (query-bias correction), h (values)
  - f = Wv @ xq + bv (queries, own half = x columns 0..2047)
  - scores s^T[j,i] = sum_c g_aug[c,j] * f_aug[c,i], K=33 augmented
    contraction ([f;1] x [g;gbv]) so s already includes the query bias.
    Key bias bk is constant over j -> cancels in softmax -> dropped.
    Value bias bq is folded into the output conv bias on the host.
  - exp on ACT, no max subtraction (|s| <~ 40 fits fp32 range)
  - num/den via one accumulating matmul with lhsT = [h^T | 1] per j-tile
  - reciprocal(den), broadcast over partitions via a PE ones-matmul
  - out = (gamma*Wo) @ (num*rden) + (gamma*(Wo@bq + bo)) + x  (bias via
    an exact-ones row in the rhs and a bias row in the weights; the
    residual reads the f32r x tile bitcast back to f32, so it is exact)
"""

import numpy as np

B, C, N = 4, 256, 4096
CR = 32
NH = N // 2          # queries per core
G = 512              # i-group width
NCORES = 8

_CACHE = {}


def build_program():
    """Build the (shared, SPMD) Bass program. Returns compiled nc."""
    import concourse.bacc as bacc
    import concourse.mybir as mybir
    from concourse.tile import TileContext

    dt = mybir.dt
    f32 = dt.float32
    f32r = dt.float32r
    Exp = mybir.ActivationFunctionType.Exp
    add = mybir.AluOpType.add
    mult = mybir.AluOpType.mult

    nc = bacc.Bacc("TRN2", target_bir_lowering=False, debug=False,
                   num_devices=NCORES)

    # --- I/O (all PE operands declared f32r; host passes fp32 bits) ---
    xin = nc.dram_tensor("xin", [C, N], f32r, kind="ExternalInput").ap()
    # packed weights: cols 0-191 wght, 192-255 wft, 256-511 wot(rows 0-63),
    # 512-543 e0, 544-575 idm(rows 0-31), 576 bv(fp32 bits)
    wpk = nc.dram_tensor("wpk", [128, 577], f32r, kind="ExternalInput").ap()
    res = nc.dram_tensor("res", [C, NH], f32, kind="ExternalOutput").ap()

    NJT = N // 128            # 32 j-tiles
    NIG = NH // G             # 4 i-groups
    SUPERS = [3, 3, 3, 3, 3, 3, 3, 3, 3, 3, 2]   # j-tiles per super (sum 32)
    assert sum(SUPERS) == NJT

    with TileContext(nc) as tc:
        with (
            tc.tile_pool(name="const", bufs=1) as cpool,
            tc.tile_pool(name="big", bufs=1) as bpool,
            tc.tile_pool(name="eb", bufs=3) as epool,
            tc.tile_pool(name="small", bufs=3) as spool,
            tc.tile_pool(name="resp", bufs=2) as rpool,
            tc.tile_pool(name="psA", bufs=1, space="PSUM") as psA,
            tc.tile_pool(name="psB", bufs=1, space="PSUM") as psB,
            tc.tile_pool(name="pso", bufs=1, space="PSUM") as pso,
            tc.tile_pool(name="pst", bufs=1, space="PSUM") as pst,
        ):
            # --- constants / weights (single DMA) ---
            wpk_t = cpool.tile([128, 577], f32r)
            nc.sync.dma_start(wpk_t[:], wpk[:])
            wght_t = wpk_t[:, 0:192]
            wft_t = wpk_t[:, 192:256]
            wot_t = wpk_t[0:64, 256:512]
            e0_t = wpk_t[:, 512:544]
            idm_t = wpk_t[0:32, 544:576]
            bv_t = wpk_t[:, 576:577].bitcast(f32)

            # --- x, rotated, chunk-major, 8 pieces (f32r view of fp32) ---
            xsb = bpool.tile([128, 2 * N], f32r)
            xr = xin.rearrange("(c p) (gp n) -> p gp c n", c=2, n=G)
            xsb_v = xsb[:].rearrange("p (c gp n) -> p gp c n", c=2, n=G)
            for gp in range(8):
                nc.sync.dma_start(xsb_v[:, gp], xr[:, gp])

            # --- activation buffers ---
            f_aug = bpool.tile([128, NH], f32r)   # rows: f(32), ones(1)
            g_aug = bpool.tile([128, N], f32r)    # rows: g(32), gbv(1)
            h_sb = bpool.tile([128, N], f32r)     # rows: h(32)
            hpt = bpool.tile([128, NJT * 33], f32r)  # [h^T | 1] per j-tile
            nc.vector.memset(f_aug[32:33, :].bitcast(f32), 1.0)

            SPOOLS = (psA, psB)
            SNAMES = ("sa", "sb")

            # --- stacked gh conv: psum rows [g(32); gbv(1); pad; h@64] ---
            def emit_gh_conv(grp):
                cps = pst.tile([128, G], f32, name="tl")
                for c in range(2):
                    nc.tensor.matmul(
                        cps[0:96, :],
                        wght_t[:, c * 96:(c + 1) * 96],
                        xsb[:, c * N + grp * G: c * N + (grp + 1) * G],
                        start=(c == 0), stop=(c == 1))
                sl = slice(grp * G, (grp + 1) * G)
                nc.vector.tensor_copy(g_aug[0:33, sl], cps[0:33, :])
                nc.vector.tensor_copy(h_sb[0:32, sl], cps[64:96, :])
                emit_gh_tps(grp)

            # transpose a group's 4 h j-tiles into hpt
            def emit_gh_tps(grp):
                tps = pst.tile([128, 128], f32r, name="tlt", tag="tl")
                for k in range(4):
                    t = 4 * grp + k
                    nc.tensor.transpose(
                        tps[:, k * 32:(k + 1) * 32],
                        h_sb[0:32, t * 128:(t + 1) * 128],
                        idm_t)
                hpt_v = hpt[:].rearrange("p (t w) -> p t w", w=33)
                nc.vector.tensor_copy(
                    hpt_v[:, 4 * grp:4 * grp + 4, 0:32],
                    tps[:].rearrange("p (t w) -> p t w", w=32))
                nc.vector.memset(hpt_v[:, 4 * grp:4 * grp + 4, 32:33].bitcast(f32), 1.0)

            # --- f conv (own query half): f = Wv @ xq + bv ---
            def emit_f_conv(fg):
                cps = pst.tile([128, G], f32, name="tl")
                for c in range(2):
                    nc.tensor.matmul(
                        cps[0:32, :],
                        wft_t[:, c * 32:(c + 1) * 32],
                        xsb[:, c * N + fg * G: c * N + (fg + 1) * G],
                        start=(c == 0), stop=(c == 1))
                nc.vector.tensor_scalar(
                    f_aug[0:32, fg * G:(fg + 1) * G], cps[0:32, :],
                    bv_t[0:32, 0:1], None, add)

            # --- main attention loop (software-pipelined) ---
            stages = []
            for g in range(NIG):
                jt = 0
                for si, nt in enumerate(SUPERS):
                    stages.append((g, si, jt, nt))
                    jt += nt
            NS = len(stages)

            po_t = {}
            sps_t = {}
            eb_t = {}
            rd_t = {}

            def emit_mm1(idx):
                g, si, jt, nt = stages[idx]
                sps = SPOOLS[idx % 2].tile([128, nt * G], f32,
                                           name=SNAMES[idx % 2])
                sps_t[idx] = sps
                for t in range(nt):
                    nc.tensor.matmul(
                        sps[:, t * G:(t + 1) * G],
                        g_aug[0:33, (jt + t) * 128:(jt + t + 1) * 128],
                        f_aug[0:33, g * G:(g + 1) * G],
                        start=True, stop=True)

            def emit_exp(idx):
                g, si, jt, nt = stages[idx]
                eb = epool.tile([128, 3 * G], f32r, name="eb")
                eb_t[idx] = eb
                nc.scalar.activation(
                    eb[:, 0:nt * G], sps_t[idx][:, 0:nt * G], Exp)

            def emit_mm2(idx):
                g, si, jt, nt = stages[idx]
                eb = eb_t.pop(idx)
                sps_t.pop(idx)
                if si == 0:
                    po_t[g] = pso.tile([128, G], f32, name="o")
                for t in range(nt):
                    nc.tensor.matmul(
                        po_t[g][0:33, :],
                        hpt[:, (jt + t) * 33:(jt + t) * 33 + 33],
                        eb[:, t * G:(t + 1) * G],
                        start=(jt + t == 0), stop=(jt + t == NJT - 1))

            def emit_tail_recip(g):
                rd = spool.tile([128, G], f32r, name="rd")
                if g < 3:
                    nc.vector.memset(rd[:].bitcast(f32), 0.0)
                with nc.allow_low_precision(reason="softmax denom"):
                    nc.vector.reciprocal(rd[0:1, :], po_t[g][32:33, :])
                rd_t[g] = rd

            def emit_tail_pe(g, k):
                po = po_t.pop(g)
                rd = rd_t.pop(g)
                bc = pst.tile([128, G], f32, name="tl")
                pf = SPOOLS[k % 2].tile([128, 1024], f32, name=SNAMES[k % 2])
                nc.tensor.matmul(bc[0:32, :], e0_t, rd[:, :],
                                 start=True, stop=True)
                bcs = spool.tile([128, G], f32r, name="bcs")
                nc.vector.tensor_copy(bcs[0:32, :], bc[0:32, :])

                att = spool.tile([128, G], f32r, name="att")
                nc.vector.tensor_tensor(att[0:32, :], po[0:32, :],
                                        bcs[0:32, :], mult)
                if g < 3:
                    nc.vector.memset(att[32:64, :].bitcast(f32), 1.0)

                # output conv (gamma*Wo + bias row) -> + x residual
                for c in range(2):
                    nc.tensor.matmul(
                        pf[:, c * G:(c + 1) * G],
                        wot_t[:, c * 128:(c + 1) * 128],
                        att[0:64, :], start=True, stop=True)
                rt = rpool.tile([128, 1024], f32, name="rt")
                nc.vector.tensor_tensor(
                    rt[:, :], pf[:, :],
                    xsb[:].bitcast(f32).rearrange("p (c m) -> p c m", c=2)
                       [:, :, g * G:(g + 1) * G], add)
                out_v = res.rearrange("(c p) (gg n) -> p gg c n",
                                      c=2, n=G)[:, g]
                nc.sync.dma_start(
                    out_v, rt[:].rearrange("p (c n) -> p c n", c=2))

            # Pipeline: mm1[k+1] issues before mm2[k]; gh-conv groups
            # trickle in between igrp-0 stages (DMA-gated anyway); tail PE
            # work is delayed one stage so the reciprocal chain never
            # stalls the PE queue head.
            convs_left = list(range(1, 8))
            f_left = list(range(1, NIG))
            pending_tail = []
            emit_gh_conv(0)
            emit_f_conv(0)
            emit_mm1(0)
            for k in range(NS):
                emit_exp(k)
                g, si, jt, nt = stages[k]
                if g == 0:
                    need = min((jt + nt + 8) // 4, 7)
                    while convs_left and convs_left[0] <= need:
                        emit_gh_conv(convs_left.pop(0))
                if f_left and si >= len(SUPERS) - 2 and f_left[0] <= g + 1:
                    emit_f_conv(f_left.pop(0))
                if k + 1 < NS:
                    emit_mm1(k + 1)
                if pending_tail and k >= pending_tail[0][1] + 1:
                    gg, kk = pending_tail.pop(0)
                    emit_tail_pe(gg, k)
                emit_mm2(k)
                if si == len(SUPERS) - 1:
                    emit_tail_recip(g)
                    pending_tail.append((g, k))
            while convs_left:
                emit_gh_conv(convs_left.pop(0))
            while f_left:
                emit_f_conv(f_left.pop(0))
            while pending_tail:
                gg, kk = pending_tail.pop(0)
                emit_tail_pe(gg, kk + 2)

    nc.compile()
    return nc


def _host_prep(Wv, bv, Wk, bk, Wq, bq, Wo, bo, gamma):
    gam = float(np.asarray(gamma).reshape(-1)[0])

    # stacked gh conv weights: rows = [Wk(32); bv@Wk(1); pad(31); Wq(32)]
    w_gh = np.zeros((96, 256), np.float32)
    w_gh[0:32] = Wk
    w_gh[32] = bv @ Wk
    w_gh[64:96] = Wq
    wght = np.zeros((128, 192), np.float32)
    for c in range(2):
        wght[:, c * 96:(c + 1) * 96] = w_gh.T[c * 128:(c + 1) * 128, :]

    wft = np.zeros((128, 64), np.float32)
    for c in range(2):
        wft[:, c * 32:(c + 1) * 32] = Wv.T[c * 128:(c + 1) * 128, :]

    # output conv lhsT rows k: k<32 -> gamma*Wo^T, k==32 -> bias row
    bof = gam * (Wo @ bq + bo)                                  # [256]
    wot = np.zeros((64, 256), np.float32)
    for c in range(2):
        wot[0:32, c * 128:(c + 1) * 128] = gam * Wo[c * 128:(c + 1) * 128, :].T
        wot[32, c * 128:(c + 1) * 128] = bof[c * 128:(c + 1) * 128]

    wpk = np.zeros((128, 577), np.float32)
    wpk[:, 0:192] = wght
    wpk[:, 192:256] = wft
    wpk[0:64, 256:512] = wot
    wpk[0, 512:544] = 1.0                      # e0: ones row
    wpk[0:32, 544:576] = np.eye(32)            # idm
    wpk[0:32, 576] = bv
    return {"wpk": wpk}


def kernel(**inputs):
    from concourse.bass_utils import run_bass_kernel_spmd

    x = np.asarray(inputs["x"], np.float32)
    consts = _host_prep(
        np.asarray(inputs["Wv"], np.float32),
        np.asarray(inputs["bv"], np.float32),
        np.asarray(inputs["Wk"], np.float32),
        np.asarray(inputs["bk"], np.float32),
        np.asarray(inputs["Wq"], np.float32),
        np.asarray(inputs["bq"], np.float32),
        np.asarray(inputs["Wo"], np.float32),
        np.asarray(inputs["bo"], np.float32),
        np.asarray(inputs["gamma"], np.float32),
    )

    if "nc" not in _CACHE:
        _CACHE["nc"] = build_program()
    nc = _CACHE["nc"]

    in_maps = []
    for core in range(NCORES):
        b, ih = core // 2, core % 2
        m = dict(consts)
        # rotate keys so this core's query half sits at columns 0..NH-1
        # (softmax is invariant to key order, so this is exact)
        m["xin"] = np.ascontiguousarray(np.roll(x[b], -ih * NH, axis=1))
        in_maps.append(m)

    r = run_bass_kernel_spmd(nc, in_maps, core_ids=list(range(NCORES)),
                             trace=False)
    out = np.empty((B, C, N), np.float32)
    for core in range(NCORES):
        b, ih = core // 2, core % 2
        out[b][:, ih * NH:(ih + 1) * NH] = r.results[core]["res"]
    return out


if __name__ == "__main__":
    nc = build_program()
    print("program built ok")
